# revision 8
# baseline (speedup 1.0000x reference)
"""Trainium2 Bass kernel for nn_FineGrainedOpLstmCellV1 (LSTM cell).

B=4096, input=1024, hidden=1024, fp32.

Strategy:
- Host side: fuse the 8 gate matmuls into one GEMM: gates = [x|h] @ [[Wx],[Wh]].
  Shard across 8 cores as 4 batch-groups x 2 hidden-column-groups
  (30 MB DMA + 8.6 GFLOP per core -- near the HBM/PE ridge).
- Per core the GEMM is computed transposed (G^T = W^T @ Xh^T) so that the
  per-gate bias and sigmoid/tanh fuse into the PSUM->SBUF eviction
  (scalar.activation with per-partition bias), then the LSTM elementwise
  tail runs on-chip. No on-chip transposes: all operands are laid out on
  the host so the contraction dim lands on SBUF partitions.
- Matmuls run as float32r (full fp32 data, relaxed-precision PE mode,
  1 cycle/row vs 4 for strict fp32; measured rel err ~7e-5).
- Weight columns are permuted host-side so each 512-col hidden block j
  holds [i_j | f_j | o_j | c_j] contiguously: one 128x512 PSUM tile per
  gate per batch-half, and chunky 256KB weight DMAs.
"""

import numpy as np

import concourse.bacc as bacc
import concourse.mybir as mybir
import concourse.tile as tile
from concourse.bass_utils import run_bass_kernel_spmd

FP = mybir.dt.float32
FPR = mybir.dt.float32r
SIG = mybir.ActivationFunctionType.Sigmoid
TANH = mybir.ActivationFunctionType.Tanh

B = 4096
IN = 1024
H = 1024
R = 4              # batch groups
C = 2              # hidden-column groups
N_CORES = R * C
BS = B // R        # 1024 batch rows per core
HSH = H // C       # 512 hidden cols per core
K = IN + H         # 2048 contraction
KT = K // 128      # 16 k-tiles
JT = HSH // 128    # 4 hidden tiles per core
NN = BS // 512     # 2 moving (batch) tiles


def _build(nc):
    xhT = nc.dram_tensor("xhT", [K, BS], FPR, kind="ExternalInput")
    wp = nc.dram_tensor("wp", [K, JT * 512], FPR, kind="ExternalInput")
    bp = nc.dram_tensor("bp", [JT * 512, 1], FP, kind="ExternalInput")
    cpT = nc.dram_tensor("cpT", [HSH, BS], FP, kind="ExternalInput")
    hT = nc.dram_tensor("hT", [HSH, BS], FP, kind="ExternalOutput")
    cT = nc.dram_tensor("cT", [HSH, BS], FP, kind="ExternalOutput")

    with tile.TileContext(nc) as tc:
        with (
            tc.tile_pool(name="xh", bufs=KT) as xh_pool,
            tc.tile_pool(name="w", bufs=6) as w_pool,
            tc.tile_pool(name="gates", bufs=2) as gate_pool,
            tc.tile_pool(name="ew", bufs=2) as ew_pool,
            tc.tile_pool(name="bias", bufs=1) as bias_pool,
            tc.tile_pool(name="psum", bufs=1, space="PSUM") as psum_pool,
        ):
            # xh tiles are loaded lazily inside j==0's k-loop so the PE can
            # start after two small DMAs instead of waiting for all 8 MB.
            xh_tiles = [None] * KT

            bias_tiles = {}
            cp_tiles = []
            for j in range(JT):
                for g in range(4):
                    bt = bias_pool.tile([128, 1], FP, tag=f"b{j}{g}", name=f"b{j}{g}")
                    nc.gpsimd.dma_start(
                        out=bt[:], in_=bp[(j * 4 + g) * 128:(j * 4 + g + 1) * 128, :]
                    )
                    bias_tiles[(j, g)] = bt
                cpt = ew_pool.tile([128, BS], FP, tag=f"cp{j}", name=f"cp{j}", bufs=1)
                nc.gpsimd.dma_start(out=cpt[:], in_=cpT[j * 128:(j + 1) * 128, :])
                cp_tiles.append(cpt)

            for j in range(JT):
                ps = [
                    [
                        psum_pool.tile([128, 512], FP, tag=f"ps{g}{n}", name=f"ps{g}{n}")
                        for n in range(NN)
                    ]
                    for g in range(4)
                ]
                for k in range(KT):
                    dma_eng = nc.sync if k % 2 == 0 else nc.scalar
                    wt = w_pool.tile([128, 512], FPR, tag="w", name=f"w{j}_{k}")
                    dma_eng.dma_start(
                        out=wt[:], in_=wp[k * 128:(k + 1) * 128, j * 512:(j + 1) * 512]
                    )
                    if j == 0:
                        t = xh_pool.tile([128, BS], FPR, tag="xh", name=f"xh{k}")
                        dma_eng.dma_start(out=t[:], in_=xhT[k * 128:(k + 1) * 128, :])
                        xh_tiles[k] = t
                    for g in range(4):
                        lhs = wt[:, g * 128:(g + 1) * 128]
                        for n in range(NN):
                            nc.tensor.matmul(
                                ps[g][n][:],
                                lhs,
                                xh_tiles[k][:, n * 512:(n + 1) * 512],
                                start=(k == 0),
                                stop=(k == KT - 1),
                            )
                gsb = []
                for g in range(4):
                    gt = gate_pool.tile([128, BS], FP, tag=f"g{g}", name=f"g{g}_{j}")
                    bt = bias_tiles[(j, g)]
                    func = SIG if g < 3 else TANH
                    for n in range(NN):
                        nc.scalar.activation(
                            gt[:, n * 512:(n + 1) * 512], ps[g][n][:], func, bias=bt[:]
                        )
                    gsb.append(gt)
                ig, fg, og, cc = gsb
                cpt = cp_tiles[j]
                # elementwise tail per batch-half so the last chunk's chain is
                # short and output DMA starts earlier
                for n in range(NN):
                    sl = slice(n * 512, (n + 1) * 512)
                    t1 = ew_pool.tile([128, 512], FP, tag=f"t1{n}", name=f"t1_{j}_{n}")
                    nc.vector.tensor_mul(t1[:], ig[:, sl], cc[:, sl])
                    ct = ew_pool.tile([128, 512], FP, tag=f"ct{n}", name=f"ct{j}_{n}")
                    nc.vector.tensor_mul(ct[:], fg[:, sl], cpt[:, sl])
                    nc.vector.tensor_add(ct[:], ct[:], t1[:])
                    tnh = ew_pool.tile([128, 512], FP, tag=f"tnh{n}", name=f"tnh{j}_{n}")
                    nc.scalar.activation(tnh[:], ct[:], TANH)
                    htl = ew_pool.tile([128, 512], FP, tag=f"ht{n}", name=f"ht{j}_{n}")
                    nc.vector.tensor_mul(htl[:], og[:, sl], tnh[:])
                    nc.gpsimd.dma_start(out=cT[j * 128:(j + 1) * 128, sl], in_=ct[:])
                    nc.gpsimd.dma_start(out=hT[j * 128:(j + 1) * 128, sl], in_=htl[:])
    return nc


_NC_CACHE = None
_last_in_maps = None


def _get_nc():
    global _NC_CACHE
    if _NC_CACHE is None:
        nc = bacc.Bacc(
            "TRN2", target_bir_lowering=False, debug=False, num_devices=N_CORES
        )
        _build(nc)
        nc.compile()
        _NC_CACHE = nc
    return _NC_CACHE


# Column permutation applied to the fused [*, 4H] gate matrices, per
# hidden-column group c2: j-major, gate-minor, so each core-local 512-wide
# block j is [i_j | f_j | o_j | c_j].
def _col_index(c2):
    idx = np.empty(4 * HSH, np.int64)
    p = 0
    for j in range(JT):
        for g in range(4):
            base = g * H + c2 * HSH + j * 128
            idx[p:p + 128] = np.arange(base, base + 128)
            p += 128
    return idx


def kernel(x, h_prev, c_prev, igx, igu, ib, fgx, fgu, fb, ogx, ogu, ob, cgx, cgu, cb):
    nc = _get_nc()

    w_full = np.vstack([
        np.concatenate([igx, fgx, ogx, cgx], axis=1),
        np.concatenate([igu, fgu, ogu, cgu], axis=1),
    ]).astype(np.float32, copy=False)              # [2048, 4096]
    b_full = np.concatenate([ib, fb, ob, cb]).astype(np.float32, copy=False)

    wps, bps = [], []
    for c2 in range(C):
        idx = _col_index(c2)
        wps.append(np.ascontiguousarray(w_full[:, idx]))
        bps.append(np.ascontiguousarray(b_full[idx]).reshape(-1, 1))

    in_maps = []
    for r in range(R):
        rs = slice(r * BS, (r + 1) * BS)
        xh_t = np.ascontiguousarray(
            np.concatenate([x[rs], h_prev[rs]], axis=1).T
        )                                           # [2048, BS]
        for c2 in range(C):
            cp_t = np.ascontiguousarray(c_prev[rs, c2 * HSH:(c2 + 1) * HSH].T)
            in_maps.append({"xhT": xh_t, "wp": wps[c2], "bp": bps[c2], "cpT": cp_t})

    global _last_in_maps
    _last_in_maps = in_maps
    res = run_bass_kernel_spmd(nc, in_maps, list(range(N_CORES))).results

    h = np.empty((B, H), np.float32)
    c = np.empty((B, H), np.float32)
    for r in range(R):
        rs = slice(r * BS, (r + 1) * BS)
        for c2 in range(C):
            cid = r * C + c2
            cs = slice(c2 * HSH, (c2 + 1) * HSH)
            h[rs, cs] = res[cid]["hT"].T
            c[rs, cs] = res[cid]["cT"].T
    return h, c
